# revision 8
# baseline (speedup 1.0000x reference)
"""Trainium2 Bass kernel for GrapherModule:
fc1+BN1 -> KNN(k=9) -> MaxRelative conv+BN+GELU -> fc2+BN -> +residual.

Single fused SPMD kernel on 8 cores. Core d handles batch b=d//4 and query
slice qoff=(d%4)*2048: its x input is np.roll(x[b], -qoff) so queries are
always local nodes 0..2047 and the program is identical on every core.

Per-core pipeline:
  A) x -> PE-transpose -> fc1 (fp32) -> pre-h (feature-major) + BN1 stats;
     AllReduce for global stats; normalize in place; bias row
     brow[n] = -0.5*||h_n||^2, split into fp16 hi+lo rows (exact to ~2^-21).
  B) per 128-query tile: scores s[q,n] = h_q.h_n + brow[n] fully in PE
     (fp32 matmul + one K=2 fp16 rank-2 bias matmul accumulated in PSUM),
     ACT copies PSUM->SBUF, self-distance masked, DVE top-8 (max +
     max_index), index wrap for the gpsimd gather via 8 one-hot selector
     matmuls (host-supplied constants), ap_gather of neighbor features
     from SBUF, DVE reduce_max over k=9 -> maxn.  No DMAs in this loop.
  C) MaxRel conv + BN (AllReduce stats) + GELU, fc2 + BN (AllReduce),
     transpose + residual -> y.
"""
import sys, os
sys.path.insert(0, '/opt/trn_rl_repo')
os.environ.setdefault('JAX_PLATFORMS', 'cpu')

import numpy as np

B, N, C = 2, 8192, 128
K = 9
NQ = 2048          # queries per core
NT = NQ // 128     # 16 query tiles per core
EPS = 1e-5

_CACHE = {}


def _build():
    import concourse.bass as bass
    import concourse.mybir as mybir
    import concourse.tile as tile
    from concourse import bacc
    from concourse.masks import make_identity

    dt = mybir.dt
    AF = mybir.ActivationFunctionType
    ALU = mybir.AluOpType
    AX = mybir.AxisListType

    nc = bacc.Bacc("TRN2", target_bir_lowering=False, debug=False,
                   enable_asserts=False, num_devices=8)

    x_own = nc.dram_tensor("x_own", [N, C], dt.float32, kind="ExternalInput")
    fc1wT_d = nc.dram_tensor("fc1wT", [C, C], dt.float32, kind="ExternalInput")
    cw1T_d = nc.dram_tensor("cw1T", [C, C], dt.float32, kind="ExternalInput")
    cw2T_d = nc.dram_tensor("cw2T", [C, C], dt.float32, kind="ExternalInput")
    fc2wT_d = nc.dram_tensor("fc2wT", [C, C], dt.float32, kind="ExternalInput")
    selT_d = nc.dram_tensor("selT", [C, 8 * 128], dt.float32, kind="ExternalInput")
    iota_d = nc.dram_tensor("iotaf", [128], dt.float32, kind="ExternalInput")
    fc1_b = nc.dram_tensor("fc1_b", [C], dt.float32, kind="ExternalInput")
    bn1_g = nc.dram_tensor("bn1_g", [C], dt.float32, kind="ExternalInput")
    bn1_b = nc.dram_tensor("bn1_b", [C], dt.float32, kind="ExternalInput")
    conv_b = nc.dram_tensor("conv_b", [C], dt.float32, kind="ExternalInput")
    bnc_g = nc.dram_tensor("bnc_g", [C], dt.float32, kind="ExternalInput")
    bnc_b = nc.dram_tensor("bnc_b", [C], dt.float32, kind="ExternalInput")
    fc2_b = nc.dram_tensor("fc2_b", [C], dt.float32, kind="ExternalInput")
    bn2_g = nc.dram_tensor("bn2_g", [C], dt.float32, kind="ExternalInput")
    bn2_b = nc.dram_tensor("bn2_b", [C], dt.float32, kind="ExternalInput")
    y = nc.dram_tensor("y", [NQ, C], dt.float32, kind="ExternalOutput")

    def col(t):  # [C] dram -> [C,1] view
        return t[:].rearrange("(c one) -> c one", one=1)

    with tile.TileContext(nc) as tc:
        wpool = tc.alloc_tile_pool(name="w", bufs=1)
        pers = tc.alloc_tile_pool(name="pers", bufs=1)
        dram = tc.alloc_tile_pool(name="dram", bufs=2, space="DRAM")

        ident = wpool.tile([128, 128], dt.float32)
        make_identity(nc, ident[:])

        def load(shape, view, tag):
            t = wpool.tile(shape, dt.float32, tag=tag)
            nc.sync.dma_start(t[:], view)
            return t

        fc1wT = load([C, C], fc1wT_d[:], "fc1wT")
        cw1T = load([C, C], cw1T_d[:], "cw1T")
        cw2T = load([C, C], cw2T_d[:], "cw2T")
        fc2wT = load([C, C], fc2wT_d[:], "fc2wT")
        sel = load([C, 8 * 128], selT_d[:], "sel")
        iotaf = load([128, 1], iota_d[:].rearrange("(p one) -> p one", one=1), "iotaf")
        fc1b = load([C, 1], col(fc1_b), "fc1b")
        bn1g = load([C, 1], col(bn1_g), "bn1g")
        bn1bb = load([C, 1], col(bn1_b), "bn1bb")
        convb = load([C, 1], col(conv_b), "convb")
        bncg = load([C, 1], col(bnc_g), "bncg")
        bncb = load([C, 1], col(bnc_b), "bncb")
        fc2b = load([C, 1], col(fc2_b), "fc2b")
        bn2g = load([C, 1], col(bn2_g), "bn2g")
        bn2bb = load([C, 1], col(bn2_b), "bn2bb")

        ones2f = wpool.tile([2, 128], dt.float32)
        nc.vector.memset(ones2f[:], 1.0)
        ones2 = wpool.tile([2, 128], dt.float16)
        nc.vector.tensor_copy(ones2[:], ones2f[:])
        mhalf = wpool.tile([128, 1], dt.float32)
        nc.vector.memset(mhalf[:], -0.5)

        # residual prefetch: res_all[p, i*128+c?] -> token-major blocks
        res_all = pers.tile([128, NQ], dt.float32)
        nc.sync.dma_start(
            res_all[:].rearrange("p (i c) -> p i c", i=NT),
            x_own[0:NQ, :].rearrange("(i p) c -> p i c", p=128))

        # ---------- AllReduce helper ----------
        def allreduce2(sump, ssqp):
            loc = pers.tile([128, 2], dt.float32, tag="arloc")
            nc.vector.reduce_sum(loc[:, 0:1], sump[:], axis=AX.X)
            nc.vector.reduce_sum(loc[:, 1:2], ssqp[:], axis=AX.X)
            bin_ = dram.tile([128, 2], dt.float32, tag="arin")
            bout = dram.tile([128, 2], dt.float32, tag="arout")
            nc.gpsimd.dma_start(bin_[:], loc[:])
            nc.gpsimd.collective_compute(
                "AllReduce", ALU.add, replica_groups=[list(range(8))],
                ins=[bin_.opt()], outs=[bout.opt()])
            tot = pers.tile([128, 2], dt.float32, tag="artot")
            nc.gpsimd.dma_start(tot[:], bout[:])
            return tot

        def bnparams(tot, gam, bet, count):
            st = pers.tile([128, 8], dt.float32, tag="bnst")
            mm, e2, vv, rr, sc, bi = (st[:, j:j + 1] for j in range(6))
            nc.vector.tensor_scalar_mul(mm, tot[:, 0:1], 1.0 / count)
            nc.vector.tensor_scalar_mul(e2, tot[:, 1:2], 1.0 / count)
            nc.vector.tensor_tensor(vv, mm, mm, op=ALU.mult)
            nc.vector.tensor_sub(vv, e2, vv)
            nc.vector.tensor_scalar(vv, vv, EPS, None, op0=ALU.add)
            nc.vector.reciprocal(rr, vv)
            nc.scalar.activation(rr, rr, AF.Sqrt)
            nc.vector.tensor_tensor(sc, rr, gam, op=ALU.mult)
            nc.vector.tensor_tensor(bi, mm, sc, op=ALU.mult)
            nc.vector.tensor_sub(bi, bet, bi)
            return sc, bi

        # ---------- Phase A: transpose x, fc1, BN1 stats ----------
        h = pers.tile([128, N], dt.float32)      # pre-h, then h (in-place norm)
        sum_p = pers.tile([128, 16], dt.float32)
        ssq_p = pers.tile([128, 16], dt.float32)

        with tc.tile_pool(name="phA", bufs=3) as phA, \
             tc.tile_pool(name="phAj", bufs=2) as phAj, \
             tc.tile_pool(name="psT", bufs=4, space="PSUM") as psT, \
             tc.tile_pool(name="psF", bufs=2, space="PSUM") as psF:
            for g in range(16):            # groups of 4 token tiles (512 tokens)
                xTr = phA.tile([128, 512], dt.float32, tag="xTr")
                for j in range(4):
                    r0 = g * 512 + j * 128
                    xt = phA.tile([128, 128], dt.float32, tag="xt")
                    nc.sync.dma_start(xt[:], x_own[r0:r0 + 128, :])
                    pxt = psT.tile([128, 128], dt.float32, tag="pT")
                    nc.tensor.transpose(pxt[:], xt[:], ident[:])
                    nc.vector.tensor_copy(xTr[:, j * 128:(j + 1) * 128], pxt[:])
                pre = psF.tile([128, 512], dt.float32, tag="pF")
                nc.tensor.matmul(pre[:], fc1wT[:], xTr[:], start=True, stop=True)
                sl = slice(g * 512, (g + 1) * 512)
                nc.scalar.activation(h[:, sl], pre[:], AF.Identity,
                                     bias=fc1b[:], accum_out=sum_p[:, g:g + 1])
                junk = phAj.tile([128, 512], dt.float32, tag="jq")
                nc.scalar.activation(junk[:], h[:, sl], AF.Square,
                                     accum_out=ssq_p[:, g:g + 1])

        sc1, bi1 = bnparams(allreduce2(sum_p, ssq_p), bn1g[:], bn1bb[:], 4 * B * N)
        nc.scalar.activation(h[:], h[:], AF.Identity, bias=bi1, scale=sc1)

        # bias row -0.5*||h_n||^2 split into fp16 hi/lo rows bhl[2, N]
        bhl = pers.tile([2, N], dt.float16)
        with tc.tile_pool(name="nx", bufs=2) as nxp, \
             tc.tile_pool(name="psN", bufs=2, space="PSUM") as psN:
            for g in range(16):
                sl = slice(g * 512, (g + 1) * 512)
                h2 = nxp.tile([128, 512], dt.float32, tag="h2")
                nc.scalar.activation(h2[:], h[:, sl], AF.Square)
                pn = psN.tile([1, 512], dt.float32, tag="pN")
                nc.tensor.matmul(pn[:], mhalf[:], h2[:], start=True, stop=True)
                nc.scalar.activation(bhl[0:1, sl], pn[:], AF.Identity)
                blt = nxp.tile([1, 512], dt.float16, tag="blt")
                nc.vector.tensor_sub(blt[:], pn[:], bhl[0:1, sl])
                nc.sync.dma_start(bhl[1:2, sl], blt[:])

        # ---------- Phase B: scores -> top-9 -> gather -> maxn ----------
        maxn = pers.tile([128, NQ], dt.float32)
        with tc.tile_pool(name="sp", bufs=2) as sp, \
             tc.tile_pool(name="smal", bufs=4) as smal, \
             tc.tile_pool(name="gth", bufs=2) as gth, \
             tc.tile_pool(name="psB", bufs=2, space="PSUM") as psB:
            for i in range(NT):
                q0 = i * 128
                s = sp.tile([128, N], dt.float32, tag="s")
                for g in range(4):
                    pg = psB.tile([128, 2048], dt.float32, tag="pg")
                    for c_ in range(4):
                        ch = slice((g * 4 + c_) * 512, (g * 4 + c_ + 1) * 512)
                        po = pg[:, c_ * 512:(c_ + 1) * 512]
                        nc.tensor.matmul(po, h[:, q0:q0 + 128], h[:, ch],
                                         start=True, stop=False)
                        nc.tensor.matmul(po, ones2[:], bhl[:, ch],
                                         start=False, stop=True)
                    nc.scalar.activation(s[:, g * 2048:(g + 1) * 2048], pg[:],
                                         AF.Identity)
                nc.gpsimd.affine_select(
                    s[:, q0:q0 + 128], s[:, q0:q0 + 128],
                    pattern=[[1, 128]], compare_op=ALU.not_equal,
                    fill=-1e30, base=0, channel_multiplier=-1)
                v8 = smal.tile([128, 8], dt.float32, tag="v8")
                nc.vector.max(v8[:], s[:])
                i8u = smal.tile([128, 8], dt.uint32, tag="i8u")
                nc.vector.max_index(i8u[:], v8[:], s[:])
                idx9f = smal.tile([128, K], dt.float32, tag="idx9f")
                nc.vector.tensor_scalar(idx9f[:, 0:1], iotaf[:], float(q0), None,
                                        op0=ALU.add)
                nc.vector.tensor_copy(idx9f[:, 1:9], i8u[:])
                # wrap: piw[p, j*9+k] = idx9f[16j + p%16, k] via one-hot matmuls
                piw = psB.tile([128, 72], dt.float32, tag="pg")
                for j in range(8):
                    nc.tensor.matmul(piw[:, j * K:(j + 1) * K],
                                     sel[:, j * 128:(j + 1) * 128], idx9f[:],
                                     start=True, stop=True)
                iw = gth.tile([128, 72], dt.int16, tag="iw")
                nc.vector.tensor_copy(iw[:], piw[:])
                gout = gth.tile([128, 8 * K * 16], dt.float32, tag="gout")
                nc.gpsimd.ap_gather(gout[:], h[:], iw[:],
                                    channels=128, num_elems=N, d=1,
                                    num_idxs=128 * K)
                nc.vector.tensor_reduce(
                    maxn[:, q0:q0 + 128],
                    gout[:].rearrange("p (j k w) -> p j w k", j=8, k=K),
                    axis=AX.X, op=ALU.max)

        # ---------- Phase C: conv + BN + GELU, fc2 + BN, residual ----------
        convpre = pers.tile([128, NQ], dt.float32)
        csum_p = pers.tile([128, 4], dt.float32)
        cssq_p = pers.tile([128, 4], dt.float32)
        with tc.tile_pool(name="cj", bufs=2) as cj, \
             tc.tile_pool(name="psC", bufs=2, space="PSUM") as psC:
            for c_ in range(4):
                sl = slice(c_ * 512, (c_ + 1) * 512)
                r2 = cj.tile([128, 512], dt.float32, tag="r2")
                nc.vector.tensor_sub(r2[:], maxn[:, sl], h[:, sl])
                pc = psC.tile([128, 512], dt.float32, tag="pc")
                nc.tensor.matmul(pc[:], cw1T[:], h[:, sl], start=True, stop=False)
                nc.tensor.matmul(pc[:], cw2T[:], r2[:], start=False, stop=True)
                nc.scalar.activation(convpre[:, sl], pc[:], AF.Identity,
                                     bias=convb[:], accum_out=csum_p[:, c_:c_ + 1])
                jq = cj.tile([128, 512], dt.float32, tag="jq")
                nc.scalar.activation(jq[:], convpre[:, sl], AF.Square,
                                     accum_out=cssq_p[:, c_:c_ + 1])

        scc, bic = bnparams(allreduce2(csum_p, cssq_p), bncg[:], bncb[:], B * N)
        g_t = pers.tile([128, NQ], dt.float32)
        nc.scalar.activation(g_t[:], convpre[:], AF.Gelu, bias=bic, scale=scc)

        f2pre = pers.tile([128, NQ], dt.float32)
        fsum_p = pers.tile([128, 4], dt.float32)
        fssq_p = pers.tile([128, 4], dt.float32)
        with tc.tile_pool(name="fj", bufs=2) as fj, \
             tc.tile_pool(name="psD", bufs=2, space="PSUM") as psD:
            for c_ in range(4):
                sl = slice(c_ * 512, (c_ + 1) * 512)
                pf = psD.tile([128, 512], dt.float32, tag="pf")
                nc.tensor.matmul(pf[:], fc2wT[:], g_t[:, sl], start=True, stop=True)
                nc.scalar.activation(f2pre[:, sl], pf[:], AF.Identity, bias=fc2b[:],
                                     accum_out=fsum_p[:, c_:c_ + 1])
                jf = fj.tile([128, 512], dt.float32, tag="jf")
                nc.scalar.activation(jf[:], f2pre[:, sl], AF.Square,
                                     accum_out=fssq_p[:, c_:c_ + 1])

        scf, bif = bnparams(allreduce2(fsum_p, fssq_p), bn2g[:], bn2bb[:], B * N)
        outfm = pers.tile([128, NQ], dt.float32)
        nc.scalar.activation(outfm[:], f2pre[:], AF.Identity, bias=bif, scale=scf)

        with tc.tile_pool(name="op", bufs=4) as op, \
             tc.tile_pool(name="psO", bufs=2, space="PSUM") as psO:
            for i in range(NT):
                q0 = i * 128
                po = psO.tile([128, 128], dt.float32, tag="po")
                nc.tensor.transpose(po[:], outfm[:, q0:q0 + 128], ident[:])
                ot = op.tile([128, 128], dt.float32, tag="ot")
                nc.vector.tensor_add(ot[:], po[:], res_all[:, q0:q0 + 128])
                nc.sync.dma_start(y[q0:q0 + 128, :], ot[:])

        for p in (dram, pers, wpool):
            p.release()

    nc.compile()
    return nc


def _host_consts():
    # one-hot wrap selectors: selT[q, j*128 + p] = 1 iff q == 16j + (p % 16)
    selT = np.zeros((C, 8 * 128), np.float32)
    for j in range(8):
        for p in range(128):
            selT[16 * j + (p % 16), j * 128 + p] = 1.0
    iotaf = np.arange(128, dtype=np.float32)
    return selT, iotaf


def kernel(**inputs):
    from concourse import bass_utils

    if 'nc' not in _CACHE:
        _CACHE['nc'] = _build()
    nc = _CACHE['nc']

    f32 = lambda a: np.ascontiguousarray(np.asarray(a), dtype=np.float32)
    x = f32(inputs['x'])
    w = {n: f32(inputs[n]) for n in
         ['fc1_b', 'bn1_g', 'bn1_b', 'conv_b', 'bnc_g', 'bnc_b',
          'fc2_b', 'bn2_g', 'bn2_b']}
    fc1_w = f32(inputs['fc1_w'])
    conv_w = f32(inputs['conv_w'])
    fc2_w = f32(inputs['fc2_w'])
    w['fc1wT'] = np.ascontiguousarray(fc1_w.T)
    w['cw1T'] = np.ascontiguousarray(conv_w[:, 0:C].T)
    w['cw2T'] = np.ascontiguousarray(conv_w[:, C:2 * C].T)
    w['fc2wT'] = np.ascontiguousarray(fc2_w.T)
    selT, iotaf = _host_consts()
    w['selT'] = selT
    w['iotaf'] = iotaf

    in_maps = []
    for d in range(8):
        b, qoff = d // 4, (d % 4) * NQ
        m = dict(w)
        m['x_own'] = np.ascontiguousarray(np.roll(x[b], -qoff, axis=0))
        in_maps.append(m)

    r = bass_utils.run_bass_kernel_spmd(nc, in_maps, core_ids=list(range(8)))
    _CACHE['last_res'] = r

    out = np.empty((B, N, C), np.float32)
    for d in range(8):
        b, qoff = d // 4, (d % 4) * NQ
        out[b, qoff:qoff + NQ] = r.results[d]['y']
    return out


# revision 11
# speedup vs baseline: 1.2795x; 1.2795x over previous
"""Trainium2 Bass kernel for GrapherModule:
fc1+BN1 -> KNN(k=9) -> MaxRelative conv+BN+GELU -> fc2+BN -> +residual.

Single fused SPMD kernel on 8 cores. Core d handles batch b=d//4 and query
slice qoff=(d%4)*2048: inputs are np.roll(x[b], -qoff) so queries are local
nodes 0..2047 and the program is identical on every core.  The host passes
x both feature-major (for fc1) and the query rows token-major (residual),
plus transposed weights and small constant tables, so the device does no
layout shuffling.

Per-core pipeline:
  A) fc1 (fp32) -> pre-h [C, N] + BN1 partial stats; AllReduce (warmed up
     by a dummy collective at kernel start); normalize in place; bias row
     -0.5*||h_n||^2 split into fp16 hi+lo rows (exact to ~2^-21).
  B) per 128-query tile: scores s[q, n] = h_q.h_n + brow[n] fully in PE
     (fp32 matmul + one K=2 fp16 bias matmul accumulated in PSUM), ACT
     copies PSUM->SBUF, self masked, DVE top-8 (max + max_index); the
     uint32 indices are split into exact bf16 planes (idx = 32a + b) and
     wrapped into the gpsimd layout with 8 one-hot bf16 selector matmuls;
     ap_gather pulls neighbor features from SBUF; DVE reduce_max over k=9
     -> maxn.  No DMAs inside this loop.
  C) MaxRel conv + BN (AllReduce stats) + GELU, fc2 + BN (AllReduce),
     transpose + residual -> y.
"""
import sys, os
sys.path.insert(0, '/opt/trn_rl_repo')
os.environ.setdefault('JAX_PLATFORMS', 'cpu')

import numpy as np

B, N, C = 2, 8192, 128
K = 9
NQ = 2048          # queries per core
NT = NQ // 128     # 16 query tiles per core
EPS = 1e-5

_CACHE = {}


def _build():
    import concourse.bass as bass
    import concourse.mybir as mybir
    import concourse.tile as tile
    from concourse import bacc
    from concourse.masks import make_identity

    dt = mybir.dt
    AF = mybir.ActivationFunctionType
    ALU = mybir.AluOpType
    AX = mybir.AxisListType

    nc = bacc.Bacc("TRN2", target_bir_lowering=False, debug=False,
                   enable_asserts=False, num_devices=8)

    xT_d = nc.dram_tensor("xT", [C, N], dt.float32, kind="ExternalInput")
    xres_d = nc.dram_tensor("x_res", [NQ, C], dt.float32, kind="ExternalInput")
    fc1wT_d = nc.dram_tensor("fc1wT", [C, C], dt.float32, kind="ExternalInput")
    cw1T_d = nc.dram_tensor("cw1T", [C, C], dt.float32, kind="ExternalInput")
    cw2T_d = nc.dram_tensor("cw2T", [C, C], dt.float32, kind="ExternalInput")
    fc2wT_d = nc.dram_tensor("fc2wT", [C, C], dt.float32, kind="ExternalInput")
    selT_d = nc.dram_tensor("selT", [C, 8 * 128], dt.bfloat16, kind="ExternalInput")
    iota_a_d = nc.dram_tensor("iota_a", [128, NT], dt.bfloat16, kind="ExternalInput")
    iota_b_d = nc.dram_tensor("iota_b", [128, 1], dt.bfloat16, kind="ExternalInput")
    fc1_b = nc.dram_tensor("fc1_b", [C], dt.float32, kind="ExternalInput")
    bn1_g = nc.dram_tensor("bn1_g", [C], dt.float32, kind="ExternalInput")
    bn1_b = nc.dram_tensor("bn1_b", [C], dt.float32, kind="ExternalInput")
    conv_b = nc.dram_tensor("conv_b", [C], dt.float32, kind="ExternalInput")
    bnc_g = nc.dram_tensor("bnc_g", [C], dt.float32, kind="ExternalInput")
    bnc_b = nc.dram_tensor("bnc_b", [C], dt.float32, kind="ExternalInput")
    fc2_b = nc.dram_tensor("fc2_b", [C], dt.float32, kind="ExternalInput")
    bn2_g = nc.dram_tensor("bn2_g", [C], dt.float32, kind="ExternalInput")
    bn2_b = nc.dram_tensor("bn2_b", [C], dt.float32, kind="ExternalInput")
    y = nc.dram_tensor("y", [NQ, C], dt.float32, kind="ExternalOutput")

    def col(t):
        return t[:].rearrange("(c one) -> c one", one=1)

    with tile.TileContext(nc) as tc:
        wpool = tc.alloc_tile_pool(name="w", bufs=1)
        pers = tc.alloc_tile_pool(name="pers", bufs=1)
        dram = tc.alloc_tile_pool(name="dram", bufs=2, space="DRAM")

        ident = wpool.tile([128, 128], dt.float32)
        make_identity(nc, ident[:])

        # ---- collective warmup: absorb CC-init + first-op latency early ----
        wub = dram.tile([128, 2], dt.float32, tag="wub")
        wuo = dram.tile([128, 2], dt.float32, tag="wuo")
        wut = wpool.tile([128, 2], dt.float32, tag="wut")
        nc.vector.memset(wut[:], 0.0)
        nc.gpsimd.dma_start(wub[:], wut[:])
        nc.gpsimd.collective_compute(
            "AllReduce", ALU.add, replica_groups=[list(range(8))],
            ins=[wub.opt()], outs=[wuo.opt()])
        nc.gpsimd.dma_start(wut[:], wuo[:])

        def load(shape, view, tag, dtype=dt.float32):
            t = wpool.tile(shape, dtype, tag=tag)
            nc.sync.dma_start(t[:], view)
            return t

        fc1wT = load([C, C], fc1wT_d[:], "fc1wT")
        cw1T = load([C, C], cw1T_d[:], "cw1T")
        cw2T = load([C, C], cw2T_d[:], "cw2T")
        fc2wT = load([C, C], fc2wT_d[:], "fc2wT")
        sel = load([C, 8 * 128], selT_d[:], "sel", dt.bfloat16)
        iota_a = load([128, NT], iota_a_d[:], "iota_a", dt.bfloat16)
        iota_b = load([128, 1], iota_b_d[:], "iota_b", dt.bfloat16)
        fc1b = load([C, 1], col(fc1_b), "fc1b")
        bn1g = load([C, 1], col(bn1_g), "bn1g")
        bn1bb = load([C, 1], col(bn1_b), "bn1bb")
        convb = load([C, 1], col(conv_b), "convb")
        bncg = load([C, 1], col(bnc_g), "bncg")
        bncb = load([C, 1], col(bnc_b), "bncb")
        fc2b = load([C, 1], col(fc2_b), "fc2b")
        bn2g = load([C, 1], col(bn2_g), "bn2g")
        bn2bb = load([C, 1], col(bn2_b), "bn2bb")

        ones2f = wpool.tile([2, 128], dt.float32)
        nc.vector.memset(ones2f[:], 1.0)
        ones2 = wpool.tile([2, 128], dt.float16)
        nc.vector.tensor_copy(ones2[:], ones2f[:])
        mhalf = wpool.tile([128, 1], dt.float32)
        nc.vector.memset(mhalf[:], -0.5)

        res_all = pers.tile([128, NQ], dt.float32)
        nc.sync.dma_start(
            res_all[:].rearrange("p (i c) -> p i c", i=NT),
            xres_d[:].rearrange("(i p) c -> p i c", p=128))

        # ---------- AllReduce helper ----------
        def allreduce2(sump, ssqp):
            loc = pers.tile([128, 2], dt.float32, tag="arloc")
            nc.vector.reduce_sum(loc[:, 0:1], sump[:], axis=AX.X)
            nc.vector.reduce_sum(loc[:, 1:2], ssqp[:], axis=AX.X)
            bin_ = dram.tile([128, 2], dt.float32, tag="arin")
            bout = dram.tile([128, 2], dt.float32, tag="arout")
            nc.gpsimd.dma_start(bin_[:], loc[:])
            nc.gpsimd.collective_compute(
                "AllReduce", ALU.add, replica_groups=[list(range(8))],
                ins=[bin_.opt()], outs=[bout.opt()])
            tot = pers.tile([128, 2], dt.float32, tag="artot")
            nc.gpsimd.dma_start(tot[:], bout[:])
            return tot

        def bnparams(tot, gam, bet, count):
            st = pers.tile([128, 8], dt.float32, tag="bnst")
            mm, e2, vv, rr, sc, bi = (st[:, j:j + 1] for j in range(6))
            nc.vector.tensor_scalar_mul(mm, tot[:, 0:1], 1.0 / count)
            nc.vector.tensor_scalar_mul(e2, tot[:, 1:2], 1.0 / count)
            nc.vector.tensor_tensor(vv, mm, mm, op=ALU.mult)
            nc.vector.tensor_sub(vv, e2, vv)
            nc.vector.tensor_scalar(vv, vv, EPS, None, op0=ALU.add)
            nc.vector.reciprocal(rr, vv)
            nc.scalar.activation(rr, rr, AF.Sqrt)
            nc.vector.tensor_tensor(sc, rr, gam, op=ALU.mult)
            nc.vector.tensor_tensor(bi, mm, sc, op=ALU.mult)
            nc.vector.tensor_sub(bi, bet, bi)
            return sc, bi

        # ---------- Phase A: fc1, BN1 stats ----------
        h = pers.tile([128, N], dt.float32)      # pre-h, then h (in-place norm)
        sum_p = pers.tile([128, 16], dt.float32)
        ssq_p = pers.tile([128, 16], dt.float32)

        with tc.tile_pool(name="phX", bufs=1) as phX, \
             tc.tile_pool(name="phAj", bufs=2) as phAj, \
             tc.tile_pool(name="psF", bufs=3, space="PSUM") as psF:
            xT = phX.tile([128, N], dt.float32)
            nc.sync.dma_start(xT[:], xT_d[:])
            for g in range(16):
                sl = slice(g * 512, (g + 1) * 512)
                pre = psF.tile([128, 512], dt.float32, tag="pF")
                nc.tensor.matmul(pre[:], fc1wT[:], xT[:, sl], start=True, stop=True)
                nc.scalar.activation(h[:, sl], pre[:], AF.Identity,
                                     bias=fc1b[:], accum_out=sum_p[:, g:g + 1])
                junk = phAj.tile([128, 512], dt.float32, tag="jq")
                nc.scalar.activation(junk[:], h[:, sl], AF.Square,
                                     accum_out=ssq_p[:, g:g + 1])

        sc1, bi1 = bnparams(allreduce2(sum_p, ssq_p), bn1g[:], bn1bb[:], 4 * B * N)
        for g in range(4):
            sl = slice(g * 2048, (g + 1) * 2048)
            nc.scalar.activation(h[:, sl], h[:, sl], AF.Identity,
                                 bias=bi1, scale=sc1)

        # bias row -0.5*||h_n||^2 split into fp16 hi/lo rows bhl[2, N]
        bhl = pers.tile([2, N], dt.float16)
        with tc.tile_pool(name="nx", bufs=2) as nxp, \
             tc.tile_pool(name="psN", bufs=2, space="PSUM") as psN:
            for g in range(16):
                sl = slice(g * 512, (g + 1) * 512)
                h2 = nxp.tile([128, 512], dt.float32, tag="h2")
                nc.scalar.activation(h2[:], h[:, sl], AF.Square)
                pn = psN.tile([1, 512], dt.float32, tag="pN")
                nc.tensor.matmul(pn[:], mhalf[:], h2[:], start=True, stop=True)
                nc.scalar.activation(bhl[0:1, sl], pn[:], AF.Identity)
                blt = nxp.tile([1, 512], dt.float16, tag="blt")
                nc.vector.tensor_sub(blt[:], pn[:], bhl[0:1, sl])
                nc.sync.dma_start(bhl[1:2, sl], blt[:])

        # ---------- Phase B: scores -> top-9 -> gather -> maxn ----------
        maxn = pers.tile([128, NQ], dt.float32)
        with tc.tile_pool(name="sp", bufs=2) as sp, \
             tc.tile_pool(name="smal", bufs=4) as smal, \
             tc.tile_pool(name="gth", bufs=2) as gth, \
             tc.tile_pool(name="psB", bufs=3, space="PSUM") as psB, \
             tc.tile_pool(name="psW", bufs=2, space="PSUM") as psW:
            for i in range(NT):
                q0 = i * 128
                s = sp.tile([128, N], dt.float32, tag="s")
                for g in range(8):
                    pg = psB.tile([128, 1024], dt.float32, tag="pg")
                    for c_ in range(2):
                        ch = slice((g * 2 + c_) * 512, (g * 2 + c_ + 1) * 512)
                        po = pg[:, c_ * 512:(c_ + 1) * 512]
                        nc.tensor.matmul(po, h[:, q0:q0 + 128], h[:, ch],
                                         start=True, stop=False)
                        nc.tensor.matmul(po, ones2[:], bhl[:, ch],
                                         start=False, stop=True)
                    nc.scalar.activation(s[:, g * 1024:(g + 1) * 1024], pg[:],
                                         AF.Identity)
                nc.gpsimd.affine_select(
                    s[:, q0:q0 + 128], s[:, q0:q0 + 128],
                    pattern=[[1, 128]], compare_op=ALU.not_equal,
                    fill=-1e30, base=0, channel_multiplier=-1)
                v8 = smal.tile([128, 8], dt.float32, tag="v8")
                nc.vector.max(v8[:], s[:])
                i8u = smal.tile([128, 8], dt.uint32, tag="i8u")
                nc.vector.max_index(i8u[:], v8[:], s[:])
                # split idx = 32a + b into exact bf16 planes; self via iota
                au = smal.tile([128, 8], dt.uint32, tag="au")
                nc.vector.tensor_scalar(au[:], i8u[:], 5, None,
                                        op0=ALU.logical_shift_right)
                bu = smal.tile([128, 8], dt.uint32, tag="bu")
                nc.vector.tensor_scalar(bu[:], i8u[:], 31, None,
                                        op0=ALU.bitwise_and)
                iab = smal.tile([128, 18], dt.bfloat16, tag="iab")
                nc.vector.tensor_copy(iab[:, 0:1], iota_a[:, i:i + 1])
                nc.vector.tensor_copy(iab[:, 1:9], au[:])
                nc.vector.tensor_copy(iab[:, 9:10], iota_b[:])
                nc.vector.tensor_copy(iab[:, 10:18], bu[:])
                # wrap via one-hot selector matmuls: piw2[p, 18j+t] over planes
                piw2 = psW.tile([128, 8 * 18], dt.float32, tag="piw2")
                for j in range(8):
                    nc.tensor.matmul(piw2[:, j * 18:(j + 1) * 18],
                                     sel[:, j * 128:(j + 1) * 128], iab[:],
                                     start=True, stop=True)
                piw2s = smal.tile([128, 8 * 18], dt.float32, tag="piw2s")
                nc.scalar.activation(piw2s[:], piw2[:], AF.Identity)
                iw = gth.tile([128, 8 * K], dt.int16, tag="iw")
                pv = piw2s[:].rearrange("p (j t) -> p j t", t=18)
                nc.vector.scalar_tensor_tensor(
                    iw[:].rearrange("p (j k) -> p j k", k=K),
                    pv[:, :, 0:9], 32.0, pv[:, :, 9:18],
                    op0=ALU.mult, op1=ALU.add)
                gout = gth.tile([128, 8 * K * 16], dt.float32, tag="gout")
                nc.gpsimd.ap_gather(gout[:], h[:], iw[:],
                                    channels=128, num_elems=N, d=1,
                                    num_idxs=128 * K)
                nc.vector.tensor_reduce(
                    maxn[:, q0:q0 + 128],
                    gout[:].rearrange("p (j k w) -> p j w k", j=8, k=K),
                    axis=AX.X, op=ALU.max)

        # ---------- Phase C: conv + BN + GELU, fc2 + BN, residual ----------
        convpre = pers.tile([128, NQ], dt.float32)
        csum_p = pers.tile([128, 4], dt.float32)
        cssq_p = pers.tile([128, 4], dt.float32)
        with tc.tile_pool(name="cj", bufs=2) as cj, \
             tc.tile_pool(name="psC", bufs=2, space="PSUM") as psC:
            for c_ in range(4):
                sl = slice(c_ * 512, (c_ + 1) * 512)
                r2 = cj.tile([128, 512], dt.float32, tag="r2")
                nc.vector.tensor_sub(r2[:], maxn[:, sl], h[:, sl])
                pc = psC.tile([128, 512], dt.float32, tag="pc")
                nc.tensor.matmul(pc[:], cw1T[:], h[:, sl], start=True, stop=False)
                nc.tensor.matmul(pc[:], cw2T[:], r2[:], start=False, stop=True)
                nc.scalar.activation(convpre[:, sl], pc[:], AF.Identity,
                                     bias=convb[:], accum_out=csum_p[:, c_:c_ + 1])
                jq = cj.tile([128, 512], dt.float32, tag="jq")
                nc.scalar.activation(jq[:], convpre[:, sl], AF.Square,
                                     accum_out=cssq_p[:, c_:c_ + 1])

        scc, bic = bnparams(allreduce2(csum_p, cssq_p), bncg[:], bncb[:], B * N)
        g_t = pers.tile([128, NQ], dt.float32)
        nc.scalar.activation(g_t[:], convpre[:], AF.Gelu, bias=bic, scale=scc)

        f2pre = pers.tile([128, NQ], dt.float32)
        fsum_p = pers.tile([128, 4], dt.float32)
        fssq_p = pers.tile([128, 4], dt.float32)
        with tc.tile_pool(name="fj", bufs=2) as fj, \
             tc.tile_pool(name="psD", bufs=2, space="PSUM") as psD:
            for c_ in range(4):
                sl = slice(c_ * 512, (c_ + 1) * 512)
                pf = psD.tile([128, 512], dt.float32, tag="pf")
                nc.tensor.matmul(pf[:], fc2wT[:], g_t[:, sl], start=True, stop=True)
                nc.scalar.activation(f2pre[:, sl], pf[:], AF.Identity, bias=fc2b[:],
                                     accum_out=fsum_p[:, c_:c_ + 1])
                jf = fj.tile([128, 512], dt.float32, tag="jf")
                nc.scalar.activation(jf[:], f2pre[:, sl], AF.Square,
                                     accum_out=fssq_p[:, c_:c_ + 1])

        scf, bif = bnparams(allreduce2(fsum_p, fssq_p), bn2g[:], bn2bb[:], B * N)
        outfm = pers.tile([128, NQ], dt.float32)
        nc.scalar.activation(outfm[:], f2pre[:], AF.Identity, bias=bif, scale=scf)

        with tc.tile_pool(name="op", bufs=4) as op, \
             tc.tile_pool(name="psO", bufs=2, space="PSUM") as psO:
            for i in range(NT):
                q0 = i * 128
                po = psO.tile([128, 128], dt.float32, tag="po")
                nc.tensor.transpose(po[:], outfm[:, q0:q0 + 128], ident[:])
                ot = op.tile([128, 128], dt.float32, tag="ot")
                nc.vector.tensor_add(ot[:], po[:], res_all[:, q0:q0 + 128])
                nc.sync.dma_start(y[q0:q0 + 128, :], ot[:])

        for p in (dram, pers, wpool):
            p.release()

    nc.compile()
    return nc


def _host_consts():
    import ml_dtypes
    # wrap selectors: selT[q, j*128 + p] = 1 iff q == 16j + (p % 16)
    selT = np.zeros((C, 8 * 128), ml_dtypes.bfloat16)
    for j in range(8):
        for p in range(128):
            selT[16 * j + (p % 16), j * 128 + p] = 1.0
    p = np.arange(128)
    iota_a = np.empty((128, NT), ml_dtypes.bfloat16)
    for t in range(NT):
        iota_a[:, t] = ((t * 128 + p) // 32).astype(np.float32)
    iota_b = (p % 32).astype(np.float32).astype(ml_dtypes.bfloat16).reshape(128, 1)
    return selT, iota_a, iota_b


def kernel(**inputs):
    from concourse import bass_utils

    if 'nc' not in _CACHE:
        _CACHE['nc'] = _build()
    nc = _CACHE['nc']

    f32 = lambda a: np.ascontiguousarray(np.asarray(a), dtype=np.float32)
    x = f32(inputs['x'])
    w = {n: f32(inputs[n]) for n in
         ['fc1_b', 'bn1_g', 'bn1_b', 'conv_b', 'bnc_g', 'bnc_b',
          'fc2_b', 'bn2_g', 'bn2_b']}
    fc1_w = f32(inputs['fc1_w'])
    conv_w = f32(inputs['conv_w'])
    fc2_w = f32(inputs['fc2_w'])
    w['fc1wT'] = np.ascontiguousarray(fc1_w.T)
    w['cw1T'] = np.ascontiguousarray(conv_w[:, 0:C].T)
    w['cw2T'] = np.ascontiguousarray(conv_w[:, C:2 * C].T)
    w['fc2wT'] = np.ascontiguousarray(fc2_w.T)
    selT, iota_a, iota_b = _host_consts()
    w['selT'] = selT
    w['iota_a'] = iota_a
    w['iota_b'] = iota_b

    in_maps = []
    for d in range(8):
        b, qoff = d // 4, (d % 4) * NQ
        xr = np.roll(x[b], -qoff, axis=0)
        m = dict(w)
        m['xT'] = np.ascontiguousarray(xr.T)
        m['x_res'] = np.ascontiguousarray(xr[0:NQ])
        in_maps.append(m)

    r = bass_utils.run_bass_kernel_spmd(nc, in_maps, core_ids=list(range(8)))
    _CACHE['last_res'] = r

    out = np.empty((B, N, C), np.float32)
    for d in range(8):
        b, qoff = d // 4, (d % 4) * NQ
        out[b, qoff:qoff + NQ] = r.results[d]['y']
    return out
